# revision 1
# baseline (speedup 1.0000x reference)
"""Trainium2 Bass kernel for DifferentiableGMM log-likelihood.

Computes  out[n] = logsumexp_k( -0.5*||(x[n]-mu[k])/s[k]||^2 - log|s[k]| + log w[k] )
for N=2,000,000 points, K=16 diagonal-covariance components, D=3.

Strategy (pure data parallel over 8 cores, 262144 padded points per core):
  The per-component Gaussian log-prob is a quadratic in x:
      lp[n,k] = sum_d A[k,d]*x[n,d]^2 + B[k,d]*x[n,d] + c_k
  computed as a matmul of per-point features F = [x^2, 1, x, 1] (fp16)
  against a block-diagonal weight matrix (8 point-groups x 16 components per
  streamed column).  Per x-tile:
    - x arrives host-padded to 4 lanes, so the DVE F-build is two clean
      contiguous-run ops (mul + copy);
    - the feature transpose into contraction-row layout runs on the DMA
      XBAR (dma_start_transpose, 4 calls/tile) - no PE transposes, no
      PSUM round-trip;
    - lp -> exp: 24 of 32 batches use the ACT table exp (bias folds c_k);
      8 of 32 use a Schraudolph bit-trick on the DVE (scale lp by 128/ln2
      into bf16-exponent units inside the matmul weights, add bias + clamp
      via one fused tensor_scalar to int16, bitcast to bf16), balancing the
      ACT and DVE engines;
    - the sum over k is a ones-matmul accumulating 16 rounds into a
      [128,512] PSUM tile, then one Ln pass writes the result.
  The device writes results in an interleaved order; the host gathers them
  back (pure indexing).  fp16 features keep max rel err ~1e-3 (ACT path)
  and ~7e-3 (Schraudolph points), well under the 2e-2 gate.
"""

import os
import numpy as np

K = 16
D = 3
EPS = 1e-6
N_CORES = 8
N_FULL = 2_000_000

# per-core tiling
T_TILES = 4                      # x-tiles per core
TILE_PTS = 128 * 512             # points per x-tile
NPC = T_TILES * TILE_PTS         # 262144 points per core
N_PAD = N_CORES * NPC            # 2097152

_compiled_cache = {}


def _build_nc(use_f32r=True):
    ablate = set(os.environ.get("GMM_ABLATE", "").split(","))
    # Force the ACT-table chooser to use the one set that holds Exp, Ln AND
    # Copy together, so no table reloads happen mid-kernel.  Other sets are
    # blanked (positions preserved: set ids index act_info.json).
    import concourse.bacc as _bacc_mod
    from concourse.hw_specs import get_activation_tables as _orig_gat
    def _only_combined(arch, __orig=_orig_gat):
        return {name: (fns if name == "natural_log_exp_and_others" else set())
                for name, fns in __orig(arch).items()}
    _bacc_mod.get_activation_tables = _only_combined
    defer_log = bool(int(os.environ.get("GMM_DEFER_LOG", "0")))
    reps = int(os.environ.get("GMM_REPS", "1"))
    tp_bufs = int(os.environ.get("GMM_TP", "3"))  # unused when xbar
    mp_bufs = int(os.environ.get("GMM_MP", "3"))
    sp_bufs = int(os.environ.get("GMM_SP", "1"))
    fdt_name = os.environ.get("GMM_FDT", "fp16")
    if os.environ.get("GMM_F32", "0") == "1":
        fdt_name = "f32"
    import concourse.bacc as bacc
    import concourse.mybir as mybir
    import concourse.tile as tile
    from concourse._compat import get_trn_type

    f32 = mybir.dt.float32
    f32r = mybir.dt.float32r
    AF = mybir.ActivationFunctionType

    mdt = f32r if use_f32r else f32
    # feature-path dtype: x / F / tp / ft / Wdiag / identity.  16-bit makes
    # the PE transposes 1.5x faster, enables DVE 2x modes, and halves the x
    # DMA.  fp16 keeps 10 mantissa bits (|x|<6, |x^2|<36 well in range).
    fdt = {"fp16": mybir.dt.float16, "bf16": mybir.dt.bfloat16,
           "f32": mdt}[fdt_name]
    # XBAR DMA transpose needs a 2-byte dtype
    xbar = (os.environ.get("GMM_XBAR", "1") == "1" and "noxbar" not in ablate
            and fdt_name != "f32")
    # Schraudolph batches: exp computed on DVE via bf16 bit-trick for this
    # many of the 32 batches (offloads the ACT engine).
    schr_n = int(os.environ.get("GMM_SCHR", "8"))
    schr_off = int(os.environ.get("GMM_SCHROFF", "0"))
    schr_set = ({(round(i * 32 / schr_n) + schr_off) % 32 for i in range(schr_n)}
                if schr_n else set())

    nc = bacc.Bacc(
        get_trn_type() or "TRN2",
        target_bir_lowering=False,
        debug=False,
        num_devices=N_CORES,
    )

    x_dram = nc.dram_tensor("x", [NPC, 4], fdt, kind="ExternalInput")
    wdiag_dram = nc.dram_tensor("wdiag", [128, 128], fdt, kind="ExternalInput")
    cvec_dram = nc.dram_tensor("cvec", [128, 1], f32, kind="ExternalInput")
    ones_dram = nc.dram_tensor("onesbig", [128, 256], mdt, kind="ExternalInput")
    ident_dram = nc.dram_tensor("ident", [128, 128], fdt, kind="ExternalInput")
    wdiag2_dram = nc.dram_tensor("wdiag2", [128, 128], fdt, kind="ExternalInput")
    cvec2_dram = nc.dram_tensor("cvec2", [128, 1], f32, kind="ExternalInput")
    out_dram = nc.dram_tensor("out", [NPC], f32, kind="ExternalOutput")

    with tile.TileContext(nc) as tc:
        with (
            tc.tile_pool(name="singles", bufs=1) as singles,
            tc.tile_pool(name="xin", bufs=int(os.environ.get("GMM_XIN", "6"))) as xin_pool,
            tc.tile_pool(name="fbig", bufs=int(os.environ.get("GMM_FB", "6"))) as f_pool,
            tc.tile_pool(name="ft", bufs=int(os.environ.get("GMM_FT", "3"))) as ft_pool,
            tc.tile_pool(name="etile", bufs=int(os.environ.get("GMM_E", "3"))) as e_pool,
            tc.tile_pool(name="osb", bufs=4) as out_pool,
            tc.tile_pool(name="tpsum", bufs=tp_bufs, space="PSUM") as tpsum_pool,
            tc.tile_pool(name="mpsum", bufs=mp_bufs, space="PSUM") as mpsum_pool,
            tc.tile_pool(name="spsum", bufs=sp_bufs, space="PSUM") as spsum_pool,
        ):
            # Constants precomputed on host.  Staged through compute-engine
            # copies so consumers' waits merge into their existing DVE/ACT
            # sem domains (matmul structs allow only ONE sync wait).
            Wdiag_st = singles.tile([128, 128], fdt)
            cvec_st = singles.tile([128, 1], f32)
            ones_st = singles.tile([128, 256], mdt)
            ident_st = singles.tile([128, 128], fdt)
            nc.sync.dma_start(Wdiag_st[:], wdiag_dram[:, :])
            nc.sync.dma_start(cvec_st[:], cvec_dram[:, :])
            nc.sync.dma_start(ones_st[:], ones_dram[:, :])
            nc.sync.dma_start(ident_st[:], ident_dram[:, :])
            Wdiag = singles.tile([128, 128], fdt)
            cvec = singles.tile([128, 1], f32)
            ones_big = singles.tile([128, 256], mdt)
            identity = singles.tile([128, 128], fdt)
            nc.vector.tensor_copy(Wdiag[:], Wdiag_st[:])
            nc.vector.tensor_copy(identity[:], ident_st[:])
            nc.scalar.copy(ones_big[:], ones_st[:])
            nc.scalar.copy(cvec[:], cvec_st[:])
            if schr_n:
                bf16dt = mybir.dt.bfloat16
                Wdiag2_st = singles.tile([128, 128], fdt)
                cvec2_st = singles.tile([128, 1], f32)
                nc.sync.dma_start(Wdiag2_st[:], wdiag2_dram[:, :])
                nc.sync.dma_start(cvec2_st[:], cvec2_dram[:, :])
                Wdiag2 = singles.tile([128, 128], fdt)
                cvec2 = singles.tile([128, 1], f32)
                ones_bf = singles.tile([128, 256], bf16dt)
                nc.vector.tensor_copy(Wdiag2[:], Wdiag2_st[:])
                nc.scalar.copy(cvec2[:], cvec2_st[:])
                nc.scalar.copy(ones_bf[:], ones_st[:])

            x_view = x_dram.ap().rearrange("(t p j) d -> t p (j d)", t=T_TILES, p=128)
            out_view = out_dram.ap().rearrange("(t p f) -> t p f", t=T_TILES, p=128)

            # ---------------- main loop ----------------
            def main_body():
              sums_tiles = []
              for t in range(T_TILES):
                  x_sb = xin_pool.tile([128, 512 * 4], fdt)
                  if "xdma" in ablate:
                      nc.sync.dma_start(x_sb[:, 0:64], x_view[t][:, 0:64])
                  else:
                      nc.sync.dma_start(x_sb[:], x_view[t])

                  # x arrives host-padded to 4 lanes [x0,x1,x2,1], so both
                  # F-build ops write clean contiguous 4-lane (8B) runs:
                  # F = [x0^2,x1^2,x2^2,1, x0,x1,x2,1]
                  F = f_pool.tile([128, 512, 8], fdt)
                  xg = x_sb[:].rearrange("p (j d) -> p j d", d=4)
                  JW = 8 if "xsq" in ablate else 512
                  MJ = 8 if "xmul" in ablate else JW
                  CJ = 8 if "xcopy" in ablate else JW
                  nc.vector.tensor_mul(F[:, 0:MJ, 0:4], xg[:, 0:MJ], xg[:, 0:MJ])
                  if os.environ.get("GMM_XCACT", "0") == "1":
                      nc.scalar.copy(F[:, 0:CJ, 4:8], xg[:, 0:CJ])
                  else:
                      nc.vector.tensor_copy(F[:, 0:CJ, 4:8], xg[:, 0:CJ])
                  Fflat = F[:].rearrange("p j c -> p (j c)")

                  if xbar:
                      # XBAR DMA batch-transposes the 32 [128,128] chunks:
                      # ftall[p, u, q] = Fflat[q, 128u + p].  xbn splits the
                      # tile into several calls for finer pipelining.
                      ftall = ft_pool.tile([128, 32, 128], fdt, tag="ftall")
                      if "xbart" in ablate:
                          nc.sync.dma_start_transpose(ftall[:, 0:1, :],
                                                      Fflat[:, 0:128])
                      else:
                          xbn = int(os.environ.get("GMM_XBN", "4"))
                          step = 32 // xbn
                          for g in range(xbn):
                              nc.sync.dma_start_transpose(
                                  ftall[:, g * step:(g + 1) * step, :],
                                  Fflat[:, g * step * 128:(g + 1) * step * 128])
                      ftflat = ftall[:].rearrange("p u q -> p (u q)")

                  sums = spsum_pool.tile([128, 512], f32)
                  sums_tiles.append(sums)
                  dualexp = "nodual" not in ablate
                  if "pair" in ablate:
                      # paired batches: one [128,1024] tpsum + one wide ft-copy
                      for pair in range(4):
                          tp2 = tpsum_pool.tile([128, 1024], fdt, tag="tp2", bufs=1)
                          for v in range(8):
                              cn = 8 * pair + v
                              nc.tensor.transpose(
                                  tp2[:, 128 * v:128 * v + 128],
                                  Fflat[:, 128 * cn:128 * cn + 128],
                                  identity[:],
                              )
                          ft2 = ft_pool.tile([128, 1024], fdt, tag="ft2")
                          nc.vector.tensor_copy(ft2[:], tp2[:])
                          for sub in range(2):
                              m2 = mpsum_pool.tile([128, 1024], f32)
                              for half in range(2):
                                  rows = slice(64 * half, 64 * half + 64)
                                  nc.tensor.matmul(
                                      m2[:, 512 * half:512 * half + 512],
                                      Wdiag[rows, :],
                                      ft2[rows, 512 * sub:512 * sub + 512],
                                      start=True, stop=True)
                              e2 = e_pool.tile([128, 1024], mdt, tag="e2")
                              nc.scalar.activation(e2[:], m2[:], AF.Exp,
                                                   bias=cvec[:], scale=1.0)
                              for half in range(2):
                                  s = 2 * (2 * pair + sub) + half
                                  nc.tensor.matmul(
                                      sums[:],
                                      ones_big[:, 120 - 8 * s:248 - 8 * s],
                                      e2[:, 512 * half:512 * half + 512],
                                      start=(s == 0), stop=(s == 15))
                      continue_batches = []
                  else:
                      continue_batches = range(8)
                  for batch in continue_batches:
                      if xbar:
                          ft = ftflat[:, 512 * batch:512 * batch + 512]
                      else:
                          tp = tpsum_pool.tile([128, 512], fdt)
                          TW = 8 if "transpose" in ablate else 128
                          for u in range(4):
                              cn = 4 * batch + u
                              nc.tensor.transpose(
                                  tp[:, 128 * u:128 * u + TW],
                                  Fflat[:, 128 * cn:128 * cn + 128],
                                  identity[:, 0:TW],
                              )
                          ftt = ft_pool.tile([128, 512], fdt)
                          FW = 64 if "ftcopy" in ablate else 512
                          nc.vector.tensor_copy(ftt[:, 0:FW], tp[:, 0:FW])
                          ft = ftt[:]
                      if dualexp:
                          # one wide m-psum (2 banks) + one exp for both halves
                          schr = (t * 8 + batch) in schr_set
                          m2 = mpsum_pool.tile([128, 1024], f32)
                          MW = 8 if "mm" in ablate else 512
                          for half in range(2):
                              rows = slice(64 * half, 64 * half + 64)
                              nc.tensor.matmul(
                                  m2[:, 512 * half:512 * half + MW],
                                  (Wdiag2 if schr else Wdiag)[rows, :],
                                  ft[rows, 0:MW],
                                  start=True, stop=True)
                          if schr:
                              # exp via bf16 bit-trick on DVE:
                              #   y = m2 + cvec2 (already scaled by 128/ln2),
                              #   clamped at 0, rounded to int16, bitcast bf16
                              e16 = e_pool.tile([128, 1024], mybir.dt.int16,
                                                tag="e16")
                              nc.vector.tensor_scalar(
                                  e16[:], m2[:], cvec2[:], 0.0,
                                  mybir.AluOpType.add, mybir.AluOpType.max)
                              e_ap = e16[:].bitcast(bf16dt)
                              ones_use = ones_bf
                          else:
                              ebf = os.environ.get("GMM_EBF", "1") == "1"
                              e2 = e_pool.tile([128, 1024],
                                               bf16dt if (ebf and schr_n) else mdt,
                                               tag="e2")
                              EW = 8 if "exp" in ablate else 1024
                              if "expcopy" in ablate:
                                  nc.scalar.activation(e2[:, 0:EW], m2[:, 0:EW],
                                                       AF.Copy)
                              else:
                                  nc.scalar.activation(e2[:, 0:EW], m2[:, 0:EW],
                                                       AF.Exp, bias=cvec[:],
                                                       scale=1.0)
                              e_ap = e2[:]
                              ones_use = ones_bf if (ebf and schr_n) else ones_big
                          OW = 8 if "ones" in ablate else 512
                          for half in range(2):
                              s = 2 * batch + half
                              nc.tensor.matmul(
                                  sums[:, 0:OW],
                                  ones_use[:, 120 - 8 * s:248 - 8 * s],
                                  e_ap[:, 512 * half:512 * half + OW],
                                  start=(s == 0), stop=(s == 15))
                          continue
                      for half in range(2):
                          rows = slice(64 * half, 64 * half + 64)
                          m_ps = mpsum_pool.tile([128, 512], f32)
                          MW = 8 if "mm" in ablate else 512
                          tpos = (64 * half, 0) if "tilepos" in ablate else None
                          nc.tensor.matmul(
                              m_ps[:, 0:MW], Wdiag[rows, :], ft[rows, 0:MW],
                              start=True, stop=True, tile_position=tpos)
                          e_sb = e_pool.tile([128, 512], mdt)
                          EW = 8 if "exp" in ablate else 512
                          efunc = AF.Copy if "expcopy" in ablate else AF.Exp
                          if "expcopy" in ablate:
                              nc.scalar.copy(e_sb[:, 0:EW], m_ps[:, 0:EW])
                          elif "nobias" in ablate:
                              nc.scalar.activation(e_sb[:, 0:EW], m_ps[:, 0:EW],
                                                   efunc)
                          elif "expf32" in ablate:
                              ef = e_pool.tile([128, 512], f32, tag="ef32")
                              nc.scalar.activation(ef[:, 0:EW], m_ps[:, 0:EW],
                                                   efunc, bias=cvec[:], scale=1.0)
                              nc.scalar.activation(e_sb[:, 0:8], m_ps[:, 0:8],
                                                   efunc, bias=cvec[:], scale=1.0)
                          else:
                              nc.scalar.activation(e_sb[:, 0:EW], m_ps[:, 0:EW],
                                                   efunc, bias=cvec[:], scale=1.0)
                          s = 2 * batch + half
                          OW = 8 if "ones" in ablate else 512
                          owin = 120 if "onesfix" in ablate else 120 - 8 * s
                          nc.tensor.matmul(
                              sums[:, 0:OW],
                              ones_big[:, owin:owin + 128],
                              e_sb[:, 0:OW],
                              start=(s == 0), stop=(s == 15))

                  if not defer_log:
                      out_sb = out_pool.tile([128, 512], f32)
                      nc.scalar.activation(out_sb[:], sums[:], AF.Ln)
                      nc.sync.dma_start(out_view[t], out_sb[:])

              if not defer_log:
                  sums_tiles = []   # logs already emitted inline
              # logs batched at the end (one act-table switch)
              for t in range(len(sums_tiles)):
                  out_sb = out_pool.tile([128, 512], f32)
                  nc.scalar.activation(out_sb[:], sums_tiles[t][:], AF.Ln)
                  if "odma" in ablate:
                      nc.sync.dma_start(out_view[t][:, 0:8], out_sb[:, 0:8])
                  else:
                      nc.sync.dma_start(out_view[t], out_sb[:])

            if reps == 1:
                main_body()
            else:
                with tc.For_i(0, reps, 1):
                    main_body()

    nc.compile()
    return nc


def _output_permutation():
    """n[l]: point index for each linear output position l (per core)."""
    tt, PP, ff = np.meshgrid(np.arange(T_TILES), np.arange(128), np.arange(512),
                             indexing="ij")
    batch, Pr = PP // 16, PP % 16
    half, b = Pr // 8, Pr % 8
    u, p = ff // 128, ff % 128
    n = (tt * 128 + p) * 512 + 64 * batch + 16 * u + 8 * half + b
    return n.reshape(-1)


def _host_constants(means, covariances, weights):
    """Wdiag [128,128], cvec [128,1], ones_big [128,256], identity [128,128],
    plus the Schraudolph-scaled wdiag2/cvec2 (bf16 bit-trick exp)."""
    covp = covariances.astype(np.float64) + EPS
    mu = means.astype(np.float64)
    A = -0.5 / covp                              # [K,D] coeff of x^2
    B = mu / covp                                # [K,D] coeff of x
    c_k = (-0.5 * (mu * mu / covp).sum(1) - 0.5 * np.log(covp).sum(1)
           - 0.5 * D * np.log(2 * np.pi) + np.log(weights.astype(np.float64)))

    def blockdiag(coefT):
        wd8 = np.zeros((64, 128), np.float32)
        for b in range(8):
            wd8[8 * b:8 * b + 8, 16 * b:16 * b + 16] = coefT
        return np.concatenate([wd8, wd8], 0)

    # feature order matches the 4-lane-padded x: [x^2(3), 1, x(3), 1]
    coefT = np.zeros((8, K), np.float64)
    coefT[0:3] = A.T
    coefT[4:7] = B.T
    wdiag = blockdiag(coefT.astype(np.float32))
    cvec = np.tile(c_k.astype(np.float32), 8).reshape(128, 1)

    # Schraudolph: y = (lp/ln2 + 127)*128 - 5.5, bitcast int16 -> bf16
    S = 128.0 / np.log(2.0)
    wdiag2 = blockdiag((coefT * S).astype(np.float32))
    c2_k = (c_k * S + 127.0 * 128.0
            - float(os.environ.get("GMM_SCHR_OFF", "5.5")))
    cvec2 = np.tile(c2_k.astype(np.float32), 8).reshape(128, 1)

    ones_big = np.zeros((128, 256), np.float32)
    for b in range(8):
        ones_big[16 * b:16 * b + 16, 120 + b] = 1.0

    ident = np.eye(128, dtype=np.float32)
    return wdiag, cvec, ones_big, ident, wdiag2, cvec2


def _prep_in_maps(x_pad, means, covariances, weights):
    """Per-core input maps, cast to the dtypes _build_nc declares."""
    wdiag, cvec, ones_big, ident, wdiag2, cvec2 = _host_constants(
        means, covariances, weights)
    fdt_name = os.environ.get("GMM_FDT", "fp16")
    if os.environ.get("GMM_F32", "0") == "1":
        fdt_name = "f32"
    if fdt_name == "fp16":
        cast = np.float16
    elif fdt_name == "bf16":
        import ml_dtypes
        cast = ml_dtypes.bfloat16
    else:
        cast = None
    # pad x to 4 lanes [x0, x1, x2, 1] so the device F-build writes clean
    # contiguous 4-lane runs
    x4 = np.empty((x_pad.shape[0], 4), dtype=x_pad.dtype)
    x4[:, 0:3] = x_pad
    x4[:, 3] = 1.0
    if cast is not None:
        x4 = x4.astype(cast)
        wdiag = wdiag.astype(cast)
        ident = ident.astype(cast)
        wdiag2 = wdiag2.astype(cast)
    in_maps = []
    for c in range(N_CORES):
        in_maps.append({
            "x": np.ascontiguousarray(x4[c * NPC:(c + 1) * NPC]),
            "wdiag": wdiag,
            "cvec": cvec,
            "onesbig": ones_big,
            "ident": ident,
            "wdiag2": wdiag2,
            "cvec2": cvec2,
        })
    return in_maps


def kernel(x, means, covariances, weights):
    from concourse.bass_utils import run_bass_kernel_spmd

    x = np.ascontiguousarray(np.asarray(x, dtype=np.float32))
    means = np.ascontiguousarray(np.asarray(means, dtype=np.float32))
    covariances = np.ascontiguousarray(np.asarray(covariances, dtype=np.float32))
    weights = np.ascontiguousarray(np.asarray(weights, dtype=np.float32)).reshape(K)

    n = x.shape[0]
    x_pad = np.zeros((N_PAD, D), dtype=np.float32)
    x_pad[:n] = x

    key = "nc"
    if key not in _compiled_cache:
        _compiled_cache[key] = _build_nc(use_f32r=True)
    nc = _compiled_cache[key]

    in_maps = _prep_in_maps(x_pad, means, covariances, weights)

    res = run_bass_kernel_spmd(
        nc, in_maps, core_ids=list(range(N_CORES)),
        trace=bool(int(os.environ.get("GMM_TRACE", "0"))),
    )
    kernel.last_results = res

    perm = _output_permutation()
    out_pad = np.empty(N_PAD, dtype=np.float32)
    for c in range(N_CORES):
        raw = res.results[c]["out"].reshape(-1)
        out_pad[c * NPC + perm] = raw
    return out_pad[:n]



# revision 2
# speedup vs baseline: 1.5455x; 1.5455x over previous
"""Trainium2 Bass kernel v2 for DifferentiableGMM log-likelihood.

Computes  out[n] = logsumexp_k( -0.5*||(x[n]-mu[k])/s[k]||^2 - log|s[k]| + log w[k] )
for N=2,000,000 points, K=16 diagonal-covariance components, D=3.

v2 strategy (vs v1): eliminate the on-device feature transpose entirely.
  The host ships x already transposed into "contraction-row" layout
  (pure layout: reshape/cast, no host compute beyond the baseline's cast):
    xt [64, 16384] fp16 per core, row 4g+d = x4[16j+g, d], j in [0,16384)
  The device builds the quadratic feature rows with ONE tensor_mul:
    ft [128, cols]: rows 0..63 = xt*xt (squares), rows 64..127 = xt
  Per-point component log-probs come from two 128-contraction matmuls
  (pass P covers components 8P..8P+7):
    m[16t+c... out[8t+c, col] = sum_d A[k,d] x_d^2 + B[k,d] x_d,  k = 8P+c
  exp with the +c_k bias runs on ACT (table exp, bias arg) for some
  (pass, block-pair) units and on DVE (Schraudolph int16 bit-trick) for
  the rest, balancing the two engines.  The k-sum is a windowed
  ones-matmul accumulating 16 rounds (8 blocks x 2 passes) into one
  [128, 512] PSUM tile; one Ln pass emits the result.
"""

import os
import numpy as np

K = 16
D = 3
EPS = 1e-6
N_CORES = 8
N_FULL = 2_000_000

T_S = 4                      # sums-tiles per core
COLS_PER_S = 4096            # 16-point columns per sums-tile
COLS = T_S * COLS_PER_S      # 16384 columns per core
NPC = COLS * 16              # 262144 points per core
N_PAD = N_CORES * NPC        # 2097152

_compiled_cache = {}


def _schr_set():
    n = int(os.environ.get("GMM2_SCHR", "13"))
    return {round(i * 32 / n) % 32 for i in range(n)} if n else set()


def _build_nc(use_f32r=True):
    # Force the ACT-table chooser to the set holding Exp, Ln AND Copy
    # together so no table reloads happen mid-kernel.
    import concourse.bacc as _bacc_mod
    from concourse.hw_specs import get_activation_tables as _orig_gat
    def _only_combined(arch, __orig=_orig_gat):
        return {name: (fns if name == "natural_log_exp_and_others" else set())
                for name, fns in __orig(arch).items()}
    _bacc_mod.get_activation_tables = _only_combined

    reps = int(os.environ.get("GMM_REPS", "1"))
    import concourse.bacc as bacc
    import concourse.mybir as mybir
    import concourse.tile as tile
    from concourse._compat import get_trn_type

    f32 = mybir.dt.float32
    fp16 = mybir.dt.float16
    bf16 = mybir.dt.bfloat16
    i16 = mybir.dt.int16
    AF = mybir.ActivationFunctionType

    schr_set = _schr_set()
    ft_bufs = int(os.environ.get("GMM2_FTB", "3"))
    e_bufs = int(os.environ.get("GMM2_EB", "6"))
    m_bufs = int(os.environ.get("GMM2_MB", "3"))
    s_bufs = int(os.environ.get("GMM2_SB", "2"))
    o_bufs = int(os.environ.get("GMM2_OB", "3"))

    nc = bacc.Bacc(
        get_trn_type() or "TRN2",
        target_bir_lowering=False,
        debug=False,
        num_devices=N_CORES,
    )

    xt_dram = nc.dram_tensor("xt", [64, COLS], fp16, kind="ExternalInput")
    w_dram = nc.dram_tensor("wmat", [128, 4, 128], fp16, kind="ExternalInput")
    cvec_dram = nc.dram_tensor("cvec", [128, 4], f32, kind="ExternalInput")
    ones_dram = nc.dram_tensor("ones16", [128, 256], bf16, kind="ExternalInput")
    out_dram = nc.dram_tensor("out", [NPC], f32, kind="ExternalOutput")

    with tile.TileContext(nc) as tc:
        with (
            tc.tile_pool(name="singles", bufs=1) as singles,
            tc.tile_pool(name="ft", bufs=ft_bufs) as ft_pool,
            tc.tile_pool(name="etile", bufs=e_bufs) as e_pool,
            tc.tile_pool(name="osb", bufs=o_bufs) as out_pool,
            tc.tile_pool(name="mpsum", bufs=m_bufs, space="PSUM") as m_pool,
            tc.tile_pool(name="spsum", bufs=s_bufs, space="PSUM") as s_pool,
        ):
            # Constants, staged through compute-engine copies so consumers'
            # waits merge into existing engine sem domains.
            w_st = singles.tile([128, 4, 128], fp16)
            cvec_st = singles.tile([128, 4], f32)
            ones_st = singles.tile([128, 256], bf16)
            nc.sync.dma_start(w_st[:], w_dram[:, :, :])
            nc.sync.dma_start(cvec_st[:], cvec_dram[:, :])
            nc.sync.dma_start(ones_st[:], ones_dram[:, :])
            wmat = singles.tile([128, 4, 128], fp16)    # [p, {W0,W1,W0s,W1s}, col]
            cvec = singles.tile([128, 4], f32)          # cols: c0, c1, c2_0, c2_1
            ones16 = singles.tile([128, 256], bf16)
            nc.vector.tensor_copy(wmat[:], w_st[:])
            nc.vector.tensor_copy(ones16[:], ones_st[:])
            nc.scalar.copy(cvec[:], cvec_st[:])

            xt_view = xt_dram.ap().rearrange("p (s c) -> s p c", s=T_S)
            out_view = out_dram.ap().rearrange("(s p f) -> s p f", s=T_S, p=128)

            LAG = int(os.environ.get("GMM2_LAG", "2"))
            LAG_LN = int(os.environ.get("GMM2_LAG_LN", "1"))
            SQ_AHEAD = int(os.environ.get("GMM2_SQA", "2"))
            NU = T_S * 8  # units per iteration

            def main_body():
                # Per-iteration state; unit u covers cols [1024u, 1024u+1024)
                # of the per-core stream: S = u//8, pass P = (u//4)%2,
                # block-pair q = u%4 -> ft cols [1024*(u%8 rotated)]...
                # Simpler: within sums-tile S, local unit v=u%8: P=v//4,
                # q=v%4 covers ft[S] cols [1024q, 1024q+1024).
                fts = {}
                e_aps = {}
                ms = {}
                sums_tiles = {}
                lns = []

                def ensure_ft(S):
                    if S in fts or S >= T_S:
                        return
                    ft = ft_pool.tile([128, COLS_PER_S], fp16)
                    nc.sync.dma_start(ft[64:128, :], xt_view[S])
                    fts[S] = ft

                def do_square(u):
                    # squares for the ft cols unit u consumes
                    if u >= NU:
                        return
                    S, v = u // 8, u % 8
                    q = v % 4
                    ensure_ft(S)
                    ft = fts[S]
                    if v // 4 == 0:  # only once per (S, q): pass 0 does it
                        nc.vector.tensor_mul(
                            ft[0:64, 1024 * q:1024 * q + 1024],
                            ft[64:128, 1024 * q:1024 * q + 1024],
                            ft[64:128, 1024 * q:1024 * q + 1024])

                def do_mm_exp(u):
                    S, v = u // 8, u % 8
                    P, q = v // 4, v % 4
                    ft = fts[S]
                    schr = u in schr_set
                    w_ap = wmat[:, (P + 2) if schr else P, :]
                    m = m_pool.tile([128, 1024], f32)
                    for h in range(2):
                        nc.tensor.matmul(
                            m[:, 512 * h:512 * h + 512],
                            w_ap,
                            ft[:, 1024 * q + 512 * h:1024 * q + 512 * h + 512],
                            start=True, stop=True)
                    if schr:
                        e16 = e_pool.tile([128, 1024], i16, tag="e16")
                        nc.vector.tensor_scalar(
                            e16[:], m[:], cvec[:, (P + 2):(P + 3)],
                            0.0, mybir.AluOpType.add, mybir.AluOpType.max)
                        e_aps[u] = e16[:].bitcast(bf16)
                    else:
                        e = e_pool.tile([128, 1024], bf16, tag="ebf")
                        nc.scalar.activation(
                            e[:], m[:], AF.Exp,
                            bias=cvec[:, P:P + 1], scale=1.0)
                        e_aps[u] = e[:]

                def do_ksum(u):
                    S, v = u // 8, u % 8
                    q = v % 4
                    if S not in sums_tiles:
                        sums_tiles[S] = [s_pool.tile([128, 512], f32,
                                                     name="sums"), 0]
                    st = sums_tiles[S]
                    e_ap = e_aps.pop(u)
                    for h in range(2):
                        blk = 2 * q + h
                        nc.tensor.matmul(
                            st[0][:],
                            ones16[:, 120 - 16 * blk:248 - 16 * blk],
                            e_ap[:, 512 * h:512 * h + 512],
                            start=(st[1] == 0), stop=(st[1] == 15))
                        st[1] += 1
                    if st[1] == 16:
                        lns.append(S)

                def do_ln(S):
                    out_sb = out_pool.tile([128, 512], f32)
                    nc.scalar.activation(out_sb[:], sums_tiles[S][0][:], AF.Ln)
                    nc.sync.dma_start(out_view[S], out_sb[:])

                FTA = int(os.environ.get("GMM2_FTA", "8"))
                ensure_ft(0)
                for w in range(SQ_AHEAD):
                    do_square(w)
                pend_ln = []
                for g in range(NU + LAG):
                    ensure_ft((g + FTA) // 8)
                    if g < NU:
                        do_square(g + SQ_AHEAD)
                        do_mm_exp(g)
                    if g >= LAG:
                        do_ksum(g - LAG)
                        while lns:
                            pend_ln.append((lns.pop(0), g))
                    while pend_ln and (g - pend_ln[0][1] >= LAG_LN
                                       or g == NU + LAG - 1):
                        do_ln(pend_ln.pop(0)[0])
                while pend_ln:
                    do_ln(pend_ln.pop(0)[0])

            if reps == 1:
                main_body()
            elif os.environ.get("GMM2_NOHWLOOP", "0") == "1":
                for _ in range(reps):
                    main_body()
            else:
                # Unroll U bodies per hardware-loop iteration: For_i ends
                # every iteration with an all-engine barrier, so adjacent
                # bodies only pipeline inside one iteration.  U amortizes
                # the barrier + pipeline fill/drain cost.
                U = int(os.environ.get("GMM2_U", "4"))
                while reps % U:
                    U -= 1
                with tc.For_i(0, reps // U, 1):
                    for _ in range(U):
                        main_body()

    nc.compile()
    return nc


def _output_permutation():
    """n[l]: point index for each linear output position l (per core)."""
    SS, qq, ff = np.meshgrid(np.arange(T_S), np.arange(128), np.arange(512),
                             indexing="ij")
    blk, t = qq // 16, qq % 16
    n = 16 * (SS * COLS_PER_S + blk * 512 + ff) + t
    return n.reshape(-1)


def _host_constants(means, covariances, weights):
    """wmat [128,4,128] (W0,W1,W0s,W1s), cvec [128,4], ones16 [128,256]."""
    covp = covariances.astype(np.float64) + EPS
    mu = means.astype(np.float64)
    A = -0.5 / covp                              # [K,D] coeff of x^2
    B = mu / covp                                # [K,D] coeff of x
    c_k = (-0.5 * (mu * mu / covp).sum(1) - 0.5 * np.log(covp).sum(1)
           - 0.5 * D * np.log(2 * np.pi) + np.log(weights.astype(np.float64)))

    Sc = 128.0 / np.log(2.0)
    schr_off = float(os.environ.get("GMM_SCHR_OFF", "5.5"))

    def build_w(P, scaled):
        w = np.zeros((128, 128), np.float64)
        for t in range(16):
            for c in range(8):
                k = 8 * P + c
                col = 8 * t + c
                w[4 * t:4 * t + 3, col] = A[k]
                w[64 + 4 * t:64 + 4 * t + 3, col] = B[k]
        if scaled:
            w = w * Sc
        return w.astype(np.float32)

    wmat = np.stack([build_w(0, False), build_w(1, False),
                     build_w(0, True), build_w(1, True)], axis=1)

    cvec = np.zeros((128, 4), np.float64)
    c2_k = c_k * Sc + 127.0 * 128.0 - schr_off
    for p in range(128):
        c = p % 8
        cvec[p, 0] = c_k[c]
        cvec[p, 1] = c_k[8 + c]
        cvec[p, 2] = c2_k[c]
        cvec[p, 3] = c2_k[8 + c]

    ones16 = np.zeros((128, 256), np.float32)
    for t in range(16):
        ones16[8 * t:8 * t + 8, 120 + t] = 1.0
    return wmat.astype(np.float32), cvec.astype(np.float32), ones16


def _prep_in_maps(x_pad, means, covariances, weights):
    """Per-core input maps. x_pad: [N_PAD, D] fp32."""
    import ml_dtypes
    wmat, cvec, ones16 = _host_constants(means, covariances, weights)
    wmat16 = wmat.astype(np.float16)
    ones_bf = ones16.astype(ml_dtypes.bfloat16)

    # host-side layout: [N_PAD, 3] -> per-core [64, COLS] fp16 where
    # row 4g+d = x4[16j+g, d] (x4 = x padded with a 4th lane of 1s)
    x4 = np.empty((N_PAD, 4), dtype=np.float16)
    x4[:, 0:3] = x_pad.astype(np.float16)
    x4[:, 3] = 1.0
    # [N_CORES, COLS, 16, 4] -> [N_CORES, 16, 4, COLS] -> [N_CORES, 64, COLS]
    xt = np.ascontiguousarray(
        x4.reshape(N_CORES, COLS, 16, 4).transpose(0, 2, 3, 1)
    ).reshape(N_CORES, 64, COLS)

    in_maps = []
    for c in range(N_CORES):
        in_maps.append({
            "xt": xt[c],
            "wmat": wmat16,
            "cvec": cvec,
            "ones16": ones_bf,
        })
    return in_maps


def kernel(x, means, covariances, weights):
    from concourse.bass_utils import run_bass_kernel_spmd

    x = np.ascontiguousarray(np.asarray(x, dtype=np.float32))
    means = np.ascontiguousarray(np.asarray(means, dtype=np.float32))
    covariances = np.ascontiguousarray(np.asarray(covariances, dtype=np.float32))
    weights = np.ascontiguousarray(np.asarray(weights, dtype=np.float32)).reshape(K)

    n = x.shape[0]
    x_pad = np.zeros((N_PAD, D), dtype=np.float32)
    x_pad[:n] = x

    key = "nc"
    if key not in _compiled_cache:
        _compiled_cache[key] = _build_nc(use_f32r=True)
    nc = _compiled_cache[key]

    in_maps = _prep_in_maps(x_pad, means, covariances, weights)

    res = run_bass_kernel_spmd(
        nc, in_maps, core_ids=list(range(N_CORES)),
        trace=bool(int(os.environ.get("GMM_TRACE", "0"))),
    )
    kernel.last_results = res

    perm = _output_permutation()
    out_pad = np.empty(N_PAD, dtype=np.float32)
    for c in range(N_CORES):
        raw = res.results[c]["out"].reshape(-1)
        out_pad[c * NPC + perm] = raw
    return out_pad[:n]
